# revision 77
# baseline (speedup 1.0000x reference)
"""4-layer GCN (DGL GraphConv norm='both' + mean pooling) on 8 trn2 NeuronCores.

Strategy (v2)
-------------
Nodes are sharded across the 8 cores in contiguous graph-aligned ranges
(dst-sharding); message tables are exchanged between layers with AllGather.

Aggregation z[v] = sum_{e: dst=v} table[src_e] is a stream of PE matmuls:
per 128-edge tile, lhsT = gathered rows [128, 64], rhs = a pure 0/1 one-hot
S[e, c] = (dstcol_e == c) built on the DVE with ONE batched tensor_tensor
is_equal over 4 tiles. All normalization/bias terms are folded algebraically:

  h_l = relu(norm_in * z_l + b) = norm_in * relu(z_l + b * (1/norm_in))
  table_{l+1} = norm_out * (h_l @ W) = (norm_in*norm_out) o (r_l @ W)

so biases enter as rank-1 (K=1) matmuls accumulated into PSUM, per-node
scales ride activation `scale=` at the node-major transpose evacuation, and
edge tiles need no per-edge weighting at all. Gathers use gpsimd.dma_gather
(int16, 4 table chunks); layer-1 messages are pre-gathered on the host.

One uniform SPMD program runs on all 8 cores: a fixed tile->column-window
schedule makes the instruction stream core-independent; all per-core
variation flows through data (indices, dstcol, pool/norm vectors).
"""
import sys
sys.path.insert(0, '/opt/trn_rl_repo')

import numpy as np
from ml_dtypes import bfloat16 as ml_bf16

import concourse.bass as bass
import concourse.bacc as bacc
import concourse.tile as tile
import concourse.mybir as mybir
from concourse import bass_utils
from concourse.masks import make_identity

F32 = mybir.dt.float32
I16 = mybir.dt.int16
BF16 = mybir.dt.bfloat16

N_CORES = 8
SBATCH = 8  # S tiles built per DVE op
NQ = 4  # SWDGE queues: gather on queue q runs on gpsimd cpu pair (2q, 2q+1)

# -------------------- host-side graph preprocessing --------------------


SEG = 512


def colbase_of(t, B, T, ncols=128):
    """Per-512-segment schedule: T = nseg*T_seg tiles; within a segment the
    window base sweeps [0, SEG-ncols] linearly (no PSUM bank straddles)."""
    ss = min(SEG, B)
    nseg = B // ss
    T_seg = T // nseg
    seg, ts = divmod(t, T_seg)
    return ss * seg + ((ss - ncols) * ts) // max(T_seg - 1, 1)


def _feasible(dst, T, B, ncols=128, cap=128):
    if len(dst) == 0:
        return True
    if len(dst) > T * cap:
        return False
    ptr, n = 0, len(dst)
    for t in range(T):
        base = colbase_of(t, B, T, ncols)
        end = base + ncols
        c = 0
        while ptr < n and c < cap and dst[ptr] < end:
            if dst[ptr] < base:
                return False
            ptr += 1
            c += 1
    return ptr == n


def _feasible_T(runs, B, ncols=128, cap=128):
    """Min per-run T (in nseg steps): runs is a list of dst lists for ONE
    run index across cores; returns smallest T covering all of them."""
    nseg = B // min(SEG, B)
    Ts = 1
    for dst in runs:
        while not _feasible(dst, Ts * nseg, B, ncols, cap):
            Ts += 1
    return Ts * nseg


def _assign_tiles(dst, B, T, ncols=128, cap=128):
    out = [[] for _ in range(T)]
    ptr, n = 0, len(dst)
    for t in range(T):
        base = colbase_of(t, B, T, ncols)
        end = base + ncols
        while ptr < n and len(out[t]) < cap and dst[ptr] < end:
            assert dst[ptr] >= base
            out[t].append(ptr)
            ptr += 1
    assert ptr == n, f"schedule infeasible T={T}"
    return out


def prep_host(x, src, dst, graph_ids, n_graphs, B, nblk):
    N = x.shape[0]
    deg_out = np.bincount(src, minlength=N).astype(np.float32)
    deg_in = np.bincount(dst, minlength=N).astype(np.float32)
    norm_out = 1.0 / np.sqrt(np.maximum(deg_out, 1.0))
    norm_in = 1.0 / np.sqrt(np.maximum(deg_in, 1.0))

    gstart = np.searchsorted(graph_ids, np.arange(n_graphs + 1))
    cuts = [0]
    for c in range(1, N_CORES):
        g = int(np.searchsorted(gstart, N * c // N_CORES))
        cuts.append(min(max(g, cuts[-1]), n_graphs))
    cuts.append(n_graphs)
    core_glo, core_ghi = cuts[:-1], cuts[1:]
    core_nlo = [int(gstart[g]) for g in core_glo]
    core_nhi = [int(gstart[g]) for g in core_ghi]

    Npad = B * nblk
    maxn = max(hi - lo for lo, hi in zip(core_nlo, core_nhi))
    assert maxn <= Npad, f"core nodes {maxn} > Npad {Npad}"

    # per-core relabel: deal nodes to blocks round-robin by in-degree rank
    # (balances block degree mass), then place randomly within the block so
    # any 128-position window sees near-uniform edge density
    rng = np.random.default_rng(12345)
    pos_global = np.zeros(N, dtype=np.int64)
    inv_pos = []
    for c in range(N_CORES):
        lo, hi = core_nlo[c], core_nhi[c]
        order = np.argsort(-deg_in[lo:hi], kind='stable')
        inv = np.full(Npad, -1, dtype=np.int64)
        blocks = [[] for _ in range(nblk)]
        for i, nd in enumerate(order):
            blocks[i % nblk].append(nd)
        for b in range(nblk):
            lst = blocks[b]
            perm = rng.permutation(B)[:len(lst)]
            for nd, p0 in zip(lst, perm):
                p = b * B + int(p0)
                pos_global[lo + nd] = p
                inv[p] = nd
        inv_pos.append(inv)

    owner = np.searchsorted(np.array(core_nhi), np.arange(N), side='right')
    dst_owner = owner[dst]

    NTOT = N_CORES * Npad
    CH = 4
    QS = Npad // CH  # quarter rows per core
    assert QS % 128 == 0, f"Npad/CH={QS} must be a multiple of 128"
    chunk_rows = NTOT // CH
    assert chunk_rows <= 32767, f"chunk_rows {chunk_rows} exceeds int16"
    # table rows are quarter-major: chunk q holds [core, pos % QS] for
    # positions in [q*QS, (q+1)*QS); local row within chunk:
    src_pos = pos_global[src]
    src_chunk = src_pos // QS
    src_lrow = owner[src] * QS + (src_pos % QS)

    cores = []
    per_core_runs = []
    for c in range(N_CORES):
        mask = dst_owner == c
        e_lrow = src_lrow[mask]
        e_dstpos = pos_global[dst[mask]]
        e_srcorig = src[mask]
        blk = e_dstpos // B
        chunk = src_chunk[mask]
        order = np.lexsort((e_dstpos, chunk, blk))
        e_lrow, e_dstpos = e_lrow[order], e_dstpos[order]
        e_srcorig = e_srcorig[order]
        blk, chunk = blk[order], chunk[order]
        runs = {}
        for b in range(nblk):
            for ch in range(CH):
                idxs = np.nonzero((blk == b) & (chunk == ch))[0]
                dl = (e_dstpos[idxs] - b * B).astype(np.int64)
                runs[(b, ch)] = (idxs, dl)
        per_core_runs.append(runs)
        cores.append(dict(glo=core_glo[c], ghi=core_ghi[c],
                          nlo=core_nlo[c], nhi=core_nhi[c],
                          e_lrow=e_lrow, e_srcorig=e_srcorig))

    nruns = nblk * CH
    # per-run tile count: max over cores for that (block, chunk) run
    T_list = []
    for b in range(nblk):
        for ch in range(CH):
            T_list.append(_feasible_T(
                [per_core_runs[c][(b, ch)][1] for c in range(N_CORES)], B))
    off = np.concatenate([[0], np.cumsum(T_list)]).astype(np.int64)
    TT = int(off[-1])

    xs = x * norm_out[:, None]  # y1 = x * norm_out (layer-1 table, host side)
    for c in range(N_CORES):
        runs = per_core_runs[c]
        c0 = cores[c]
        idx16 = np.zeros(TT * 128, np.int16)
        dstcol = np.full(TT * 128, -1.0, np.float32)
        msrc = np.zeros(TT * 128, np.int64)
        mreal = np.zeros(TT * 128, bool)
        for b in range(nblk):
            for ch in range(CH):
                ridx = b * CH + ch
                T_r = T_list[ridx]
                o = int(off[ridx])
                eidx, dl = runs[(b, ch)]
                tiles = _assign_tiles(dl, B, T_r)
                for t, members in enumerate(tiles):
                    cb = colbase_of(t, B, T_r)
                    for k, m in enumerate(members):
                        s = (o + t) * 128 + k
                        ge = eidx[m]
                        idx16[s] = c0['e_lrow'][ge]
                        dstcol[s] = dl[m] - cb
                        msrc[s] = c0['e_srcorig'][ge]
                        mreal[s] = True

        idx_dev = np.zeros((128, TT * 8), np.int16)
        for r in range(nruns):
            o, T_r = int(off[r]), T_list[r]
            a = idx16[o * 128:(o + T_r) * 128].reshape(T_r * 8, 16).T
            idx_dev[:, o * 8:(o + T_r) * 8] = np.tile(a, (8, 1))
        c0['idx_dev'] = idx_dev
        c0['dstcol_dev'] = np.ascontiguousarray(
            dstcol.reshape(TT, 128).T).astype(ml_bf16)
        msg = xs[msrc].astype(np.float32)
        msg[~mreal, :] = 0.0
        msg = msg.astype(ml_bf16).reshape(TT, 128, 64).transpose(1, 0, 2)
        c0['msg1_dev'] = np.ascontiguousarray(msg.reshape(128, TT * 64))

        # per-position vectors (position space, pads harmless)
        inv = inv_pos[c]
        lo = c0['nlo']
        ni_pos = np.ones(Npad, np.float32)
        no_pos = np.zeros(Npad, np.float32)
        valid = inv >= 0
        ni_pos[valid] = norm_in[lo + inv[valid]]
        no_pos[valid] = norm_out[lo + inv[valid]]
        c0['invni_dev'] = (1.0 / ni_pos).reshape(1, Npad)
        c0['nio_dev'] = np.ascontiguousarray(
            (ni_pos * no_pos).reshape(Npad // 128, 128).T)  # [128, Npad/128]
        # pooling matrix with norm_in folded: P'[p, g] = norm_in[p]/n_g
        P = np.zeros((Npad, 128), np.float32)
        counts = (gstart[1:] - gstart[:-1]).astype(np.float64)
        gl = c0['glo']
        pidx = np.nonzero(valid)[0]
        gg = graph_ids[lo + inv[pidx]]
        P[pidx, gg - gl] = ni_pos[pidx] / np.maximum(counts[gg], 1.0)
        c0['pool_dev'] = P
    return dict(T_list=T_list, CH=CH, chunk_rows=chunk_rows, Npad=Npad,
                NTOT=NTOT, cores=cores)


# -------------------- device program --------------------


def build_program(T_list, CH, chunk_rows, Npad, NTOT, B, nblk):
    W_IN, W1, W2, W3, W4 = 64, 128, 64, 32, 4
    nruns = nblk * CH
    njt = B // 128  # node-major tiles per block
    off = [0]
    for t in T_list:
        off.append(off[-1] + t)
    TT = off[-1]
    Tmax = max(T_list)
    nc = bacc.Bacc("TRN2", target_bir_lowering=False, debug=False,
                   num_devices=N_CORES, num_swdge_queues=NQ)

    msg1_d = nc.dram_tensor("msg1", [128, TT * 64], BF16, kind="ExternalInput")
    idx_d = nc.dram_tensor("idx", [128, TT * 8], I16, kind="ExternalInput")
    dstcol_d = nc.dram_tensor("dstcol", [128, TT], BF16, kind="ExternalInput")
    w1_d = nc.dram_tensor("W1", [W_IN, W1], BF16, kind="ExternalInput")
    w2_d = nc.dram_tensor("W2", [W1, W2], BF16, kind="ExternalInput")
    w3_d = nc.dram_tensor("W3", [W2, W3], BF16, kind="ExternalInput")
    w4_d = nc.dram_tensor("W4", [W3, W4], BF16, kind="ExternalInput")
    b1_d = nc.dram_tensor("b1", [1, W1], BF16, kind="ExternalInput")
    b2_d = nc.dram_tensor("b2", [1, W2], BF16, kind="ExternalInput")
    b3_d = nc.dram_tensor("b3", [1, W3], BF16, kind="ExternalInput")
    b4_d = nc.dram_tensor("b4", [1, W4], BF16, kind="ExternalInput")
    invni_d = nc.dram_tensor("invni", [1, Npad], BF16, kind="ExternalInput")
    nio_d = nc.dram_tensor("nio", [128, Npad // 128], F32, kind="ExternalInput")
    pool_d = nc.dram_tensor("pool", [Npad, 128], BF16, kind="ExternalInput")
    out_d = nc.dram_tensor("out", [128, W4], F32, kind="ExternalOutput")

    with tile.TileContext(nc) as tc:
        import contextlib
        ctx = contextlib.ExitStack()
        with ctx:
            const = ctx.enter_context(tc.tile_pool(name="const", bufs=1))
            meta = ctx.enter_context(tc.tile_pool(name="meta", bufs=1))
            slabs = ctx.enter_context(tc.tile_pool(name="slabs", bufs=2 * CH + 1))
            spool = ctx.enter_context(tc.tile_pool(name="spool", bufs=3))
            hpool = ctx.enter_context(tc.tile_pool(name="hpool", bufs=1))
            nmpool = ctx.enter_context(tc.tile_pool(name="nmpool", bufs=3))
            ppool = ctx.enter_context(tc.tile_pool(name="ppool", bufs=3))
            psum_agg = ctx.enter_context(tc.tile_pool(name="psA", bufs=2, space="PSUM"))
            psum_w = ctx.enter_context(tc.tile_pool(name="psW", bufs=1, space="PSUM"))
            psum_tr = ctx.enter_context(tc.tile_pool(name="psT", bufs=2, space="PSUM"))
            psum_pool = ctx.enter_context(tc.tile_pool(name="psP", bufs=1, space="PSUM"))
            dram = ctx.enter_context(tc.tile_pool(name="dram", bufs=1, space="DRAM"))

            iota_i = const.tile([128, SBATCH * 128], mybir.dt.int32)
            nc.gpsimd.iota(iota_i[:], pattern=[[0, SBATCH], [1, 128]], base=0,
                           channel_multiplier=0)
            iota = const.tile([128, SBATCH * 128], BF16)
            nc.vector.tensor_copy(iota[:], iota_i[:])
            zerosb = const.tile([1, 512], BF16)
            nc.vector.memset(zerosb[:], 0.0)
            ident_f = const.tile([128, 128], F32)
            make_identity(nc, ident_f[:])
            ident = const.tile([128, 128], BF16)
            nc.vector.tensor_copy(ident[:], ident_f[:])

            w1_t = const.tile([W_IN, W1], BF16)
            w2_t = const.tile([W1, W2], BF16)
            w3_t = const.tile([W2, W3], BF16)
            w4_t = const.tile([W3, W4], BF16)
            for wt, wd in ((w1_t, w1_d), (w2_t, w2_d), (w3_t, w3_d), (w4_t, w4_d)):
                nc.sync.dma_start(out=wt[:], in_=wd[:, :])
            b1_t = const.tile([1, W1], BF16)
            b2_t = const.tile([1, W2], BF16)
            b3_t = const.tile([1, W3], BF16)
            b4_t = const.tile([1, W4], BF16)
            for bt, bd in ((b1_t, b1_d), (b2_t, b2_d), (b3_t, b3_d), (b4_t, b4_d)):
                nc.sync.dma_start(out=bt[:], in_=bd[:, :])
            invni_t = const.tile([1, Npad], BF16)
            nc.sync.dma_start(out=invni_t[:], in_=invni_d[:, :])
            nio_t = const.tile([128, Npad // 128], F32)
            nc.sync.dma_start(out=nio_t[:], in_=nio_d[:, :])

            idx_t = meta.tile([128, TT * 8], I16)
            nc.sync.dma_start(out=idx_t[:], in_=idx_d[:, :])
            dstcol_t = meta.tile([128, TT], BF16)
            nc.sync.dma_start(out=dstcol_t[:], in_=dstcol_d[:, :])

            QS = Npad // CH
            TW = (W2, W3, W3)  # true data widths of the three tables
            tables = [[dram.tile([N_CORES * QS, 128], BF16,
                                 tag=f"table{i}_{q}", name=f"table{i}_{q}",
                                 addr_space="Shared")
                       for q in range(CH)] for i in range(3)]
            tloc = [[dram.tile([QS, 128], BF16, tag=f"tloc{i}_{q}",
                               name=f"tloc{i}_{q}")
                     for q in range(CH)] for i in range(3)]

            def emit_ag(i):
                for q in range(CH):
                    nc.gpsimd.collective_compute(
                        "AllGather", mybir.AluOpType.bypass,
                        ins=[tloc[i][q].opt()], outs=[tables[i][q].opt()],
                        replica_groups=[list(range(N_CORES))])

            ACT = mybir.ActivationFunctionType
            AL = mybir.AluOpType

            def agg_layer(get_slab, bias_row, bias_w, evac, lw=128):
                """Aggregate one layer; psum gets raw z (+ bias*invni if
                bias_row); evac(b, ps). lw = slab slot width."""
                for b in range(nblk):
                    slab_tiles = [get_slab(b, chv) for chv in range(CH)]
                    ps = psum_agg.tile([128, B], F32)
                    o = 0
                    while o < B:
                        n = min(512, B - o)
                        nc.tensor.matmul(out=ps[:, o:o + n], lhsT=zerosb[:1, :128],
                                         rhs=zerosb[:1, :n], start=True, stop=False,
                                         skip_group_check=True)
                        o += n
                    for chv in range(CH):
                        slab = slab_tiles[chv]
                        ridx = b * CH + chv
                        T_r = T_list[ridx]
                        for t0 in range(0, T_r, SBATCH):
                            nb = min(SBATCH, T_r - t0)
                            col = off[ridx] + t0
                            s_t = spool.tile([128, SBATCH * 128], BF16, tag="S")
                            dc = dstcol_t[:, col:col + nb]
                            dc = dc.rearrange("p (n o) -> p n o", o=1)
                            nc.vector.tensor_tensor(
                                out=s_t[:, :nb * 128].rearrange(
                                    "p (n c) -> p n c", c=128),
                                in0=iota[:, :nb * 128].rearrange(
                                    "p (n c) -> p n c", c=128),
                                in1=dc.to_broadcast([128, nb, 128]),
                                op=AL.is_equal)
                            for ti in range(nb):
                                t = t0 + ti
                                cb = colbase_of(t, B, T_r)
                                last = (chv == CH - 1) and (t == T_r - 1) \
                                    and bias_row is None
                                m = min(128, 512 - cb % 512)
                                nc.tensor.matmul(
                                    out=ps[:lw, cb:cb + m],
                                    lhsT=slab[:, t * lw:(t + 1) * lw],
                                    rhs=s_t[:, ti * 128:ti * 128 + m],
                                    start=False, stop=last and m == 128,
                                    skip_group_check=True)
                                if m < 128:
                                    nc.tensor.matmul(
                                        out=ps[:lw, cb + m:cb + 128],
                                        lhsT=slab[:, t * lw:(t + 1) * lw],
                                        rhs=s_t[:, ti * 128 + m:(ti + 1) * 128],
                                        start=False, stop=last,
                                        skip_group_check=True)
                    if bias_row is not None:
                        o = 0
                        while o < B:
                            n = min(512, B - o)
                            lastc = o + n >= B
                            nc.tensor.matmul(out=ps[:bias_w, o:o + n],
                                             lhsT=bias_row,
                                             rhs=invni_t[:, b * B + o:b * B + o + n],
                                             start=False, stop=lastc,
                                             skip_group_check=True)
                            o += n
                    evac(b, ps)

            def transpose_scale_store(b, vT, w, tab_idx):
                """vT [w, B] FM -> node-major [128, 128] tiles scaled by nio
                -> tloc[tab_idx] (cols w: stay stale junk; they only ever land
                in unused psum rows downstream), one DMA per chunk-run."""
                nmw = nmpool.tile([128, njt * 128], BF16, tag="nmw")
                for j in range(njt):
                    pt = psum_tr.tile([128, 64], BF16, tag="pt")
                    nc.tensor.transpose(out=pt[:, :w],
                                        in_=vT[:w, j * 128:(j + 1) * 128],
                                        identity=ident[:w, :w])
                    jg = b * njt + j
                    nc.scalar.activation(nmw[:, j * 128:j * 128 + w], pt[:, :w],
                                         ACT.Copy, scale=nio_t[:, jg:jg + 1])
                j0 = 0
                while j0 < njt:
                    row = b * B + j0 * 128
                    q, qr = divmod(row, QS)
                    jn = min(njt - j0, (QS - qr) // 128)
                    nc.sync.dma_start(
                        out=tloc[tab_idx][q][qr:qr + jn * 128, :].rearrange(
                            "(j p) c -> p j c", p=128),
                        in_=nmw[:, j0 * 128:(j0 + jn) * 128].rearrange(
                            "p (j c) -> p j c", c=128))
                    j0 += jn

            def fm_matmul(rT, w_in, w_out, w_tile, bias_row, act, out_tag, b):
                """out = act(W^T @ rT [+ bias x invni]) -> [w_out, B] sbuf."""
                oT = hpool.tile([w_out, B], BF16, tag=out_tag, name=f"oT_{out_tag}")
                o = 0
                while o < B:
                    n = min(512, B - o)
                    pw = psum_w.tile([w_out, 512], F32, tag="pw")
                    nc.tensor.matmul(out=pw[:, :n], lhsT=w_tile[:],
                                     rhs=rT[:w_in, o:o + n], start=True,
                                     stop=bias_row is None, skip_group_check=True)
                    if bias_row is not None:
                        nc.tensor.matmul(out=pw[:, :n], lhsT=bias_row,
                                         rhs=invni_t[:, b * B + o:b * B + o + n],
                                         start=False, stop=True,
                                         skip_group_check=True)
                    nc.scalar.activation(oT[:, o:o + n], pw[:, :n], act)
                    o += n
                return oT

            # ---------------- Layer 1 ----------------
            def l1_slab(b, chv):
                ridx = b * CH + chv
                T_r = T_list[ridx]
                sl = slabs.tile([128, Tmax * 128], BF16, tag="slab")
                nc.sync.dma_start(
                    out=sl[:, :T_r * 64],
                    in_=msg1_d[:, off[ridx] * 64:(off[ridx] + T_r) * 64])
                return sl

            def l1_evac(b, ps):
                z1T = hpool.tile([64, B], BF16, tag="zT")
                nc.scalar.activation(z1T[:], ps[:64, :], ACT.Copy)
                r1 = fm_matmul(z1T, 64, W1, w1_t, b1_t[:1, :], ACT.Relu, "r", b)
                v2 = fm_matmul(r1, W1, W2, w2_t, None, ACT.Copy, "v", b)
                transpose_scale_store(b, v2, W2, 0)

            agg_layer(l1_slab, None, 0, l1_evac, lw=64)
            emit_ag(0)

            def gather_slab(tab, b, chv):
                """Split each chunk's gather into two half-size gathers on
                different SWDGE queues: halves generate descriptors on
                disjoint gpsimd cpu pairs concurrently."""
                ridx = b * CH + chv
                T_r = T_list[ridx]
                sl = slabs.tile([128, Tmax * 128], BF16, tag="slab")
                th = T_r // 2
                for h, (t0, t1) in enumerate(((0, th), (th, T_r))):
                    nt = t1 - t0
                    o8 = (off[ridx] + t0) * 8
                    nc.gpsimd.dma_gather(
                        out_ap=sl[:, t0 * 128:t1 * 128].rearrange(
                            "p (s d) -> p s d", d=128),
                        in_ap=tab[chv][:, :],
                        idxs_ap=idx_t[:, o8:o8 + nt * 8],
                        num_idxs=nt * 128, num_idxs_reg=nt * 128,
                        elem_size=128, single_packet=False,
                        queue_num=(2 * chv + h) % NQ)
                return sl

            # ---------------- Layer 2 ----------------
            def l2_evac(b, ps):
                r2 = hpool.tile([W2, B], BF16, tag="r", name="r2")
                nc.scalar.activation(r2[:], ps[:W2, :], ACT.Relu)
                v3 = fm_matmul(r2, W2, W3, w3_t, None, ACT.Copy, "v", b)
                transpose_scale_store(b, v3, W3, 1)

            agg_layer(lambda b, chv: gather_slab(tables[0], b, chv),
                      b2_t[:1, :], W2, l2_evac)
            emit_ag(1)

            # ---------------- Layer 3 ----------------
            def l3_evac(b, ps):
                r3 = hpool.tile([W3, B], BF16, tag="r", name="r3")
                nc.scalar.activation(r3[:], ps[:W3, :], ACT.Relu)
                transpose_scale_store(b, r3, W3, 2)

            agg_layer(lambda b, chv: gather_slab(tables[1], b, chv),
                      b3_t[:1, :], W3, l3_evac)
            emit_ag(2)

            # ---------------- Layer 4 + pooling ----------------
            pp = psum_pool.tile([128, W4], F32)

            def l4_evac(b, ps):
                z4T = hpool.tile([W3, B], BF16, tag="zT", name="z4T")
                nc.scalar.activation(z4T[:], ps[:W3, :], ACT.Copy)
                r4 = fm_matmul(z4T, W3, W4, w4_t, b4_t[:1, :], ACT.Copy, "r", b)
                pmw = ppool.tile([128, njt * 128], BF16, tag="poolmat")
                nc.sync.dma_start(
                    out=pmw[:].rearrange("p (j c) -> p j c", c=128),
                    in_=pool_d[b * B:(b + 1) * B, :].rearrange(
                        "(j p) c -> p j c", p=128))
                for j in range(njt):
                    pt = psum_tr.tile([128, 64], BF16, tag="pt")
                    nc.tensor.transpose(out=pt[:, :W4],
                                        in_=r4[:W4, j * 128:(j + 1) * 128],
                                        identity=ident[:W4, :W4])
                    nm = nmpool.tile([128, 64], BF16, tag="nm4")
                    nc.scalar.activation(nm[:, :W4], pt[:, :W4], ACT.Copy)
                    jg = b * njt + j
                    nc.tensor.matmul(out=pp[:], lhsT=pmw[:, j * 128:(j + 1) * 128],
                                     rhs=nm[:, :W4],
                                     start=(jg == 0), stop=(jg == nblk * njt - 1),
                                     skip_group_check=True)

            agg_layer(lambda b, chv: gather_slab(tables[2], b, chv),
                      None, 0, l4_evac)

            outp = ppool.tile([128, W4], F32, tag="outp")
            nc.scalar.activation(outp[:], pp[:], ACT.Copy)
            nc.sync.dma_start(out=out_d[:, :], in_=outp[:])

    nc.compile()
    return nc


# -------------------- top-level kernel --------------------


def _run(x, W1, b1, W2, b2, W3, b3, W4, b4, src, dst, graph_ids,
         n_graphs, B, nblk):
    x = np.asarray(x, np.float32)
    src = np.asarray(src, np.int64)
    dst = np.asarray(dst, np.int64)
    graph_ids = np.asarray(graph_ids, np.int64)
    H = prep_host(x, src, dst, graph_ids, n_graphs, B, nblk)
    import os as _os
    if _os.environ.get('GCN_DEBUG'):
        TT = sum(H['T_list'])
        print(f"[kernel] TT={TT} slots={TT * 128} Tmax={max(H['T_list'])} "
              f"Npad={H['Npad']} NTOT={H['NTOT']}")
    nc = build_program(H['T_list'], H['CH'], H['chunk_rows'], H['Npad'],
                       H['NTOT'], B, nblk)
    in_maps = []
    for c in range(N_CORES):
        c0 = H['cores'][c]
        in_maps.append({
            "msg1": c0['msg1_dev'],
            "idx": c0['idx_dev'],
            "dstcol": c0['dstcol_dev'],
            "W1": np.asarray(W1, np.float32).astype(ml_bf16),
            "W2": np.asarray(W2, np.float32).astype(ml_bf16),
            "W3": np.asarray(W3, np.float32).astype(ml_bf16),
            "W4": np.asarray(W4, np.float32).astype(ml_bf16),
            "b1": np.asarray(b1, np.float32).reshape(1, -1).astype(ml_bf16),
            "b2": np.asarray(b2, np.float32).reshape(1, -1).astype(ml_bf16),
            "b3": np.asarray(b3, np.float32).reshape(1, -1).astype(ml_bf16),
            "b4": np.asarray(b4, np.float32).reshape(1, -1).astype(ml_bf16),
            "invni": c0['invni_dev'].astype(ml_bf16),
            "nio": c0['nio_dev'],
            "pool": c0['pool_dev'].astype(ml_bf16),
        })
    res = bass_utils.run_bass_kernel_spmd(
        nc, in_maps, core_ids=list(range(N_CORES)),
        trace=bool(int(__import__('os').environ.get('GCN_TRACE', '0'))))
    out = np.zeros((n_graphs, 4), np.float32)
    for c in range(N_CORES):
        c0 = H['cores'][c]
        g0, g1 = c0['glo'], c0['ghi']
        out[g0:g1] = res.results[c]["out"][:g1 - g0, :]
    _run.last_exec_ns = res.exec_time_ns
    _run.last_res = res
    return out


def kernel(x, W1, b1, W2, b2, W3, b3, W4, b4, src, dst, graph_ids):
    return _run(x, W1, b1, W2, b2, W3, b3, W4, b4, src, dst, graph_ids,
                n_graphs=500, B=1024, nblk=14)



# revision 86
# speedup vs baseline: 1.2115x; 1.2115x over previous
"""4-layer GCN (DGL GraphConv norm='both' + mean pooling) on 8 trn2 NeuronCores.

Strategy (v2)
-------------
Nodes are sharded across the 8 cores in contiguous graph-aligned ranges
(dst-sharding); message tables are exchanged between layers with AllGather.

Aggregation z[v] = sum_{e: dst=v} table[src_e] is a stream of PE matmuls:
per 128-edge tile, lhsT = gathered rows [128, 64], rhs = a pure 0/1 one-hot
S[e, c] = (dstcol_e == c) built on the DVE with ONE batched tensor_tensor
is_equal over 4 tiles. All normalization/bias terms are folded algebraically:

  h_l = relu(norm_in * z_l + b) = norm_in * relu(z_l + b * (1/norm_in))
  table_{l+1} = norm_out * (h_l @ W) = (norm_in*norm_out) o (r_l @ W)

so biases enter as rank-1 (K=1) matmuls accumulated into PSUM, per-node
scales ride activation `scale=` at the node-major transpose evacuation, and
edge tiles need no per-edge weighting at all. Gathers use gpsimd.dma_gather
(int16, 4 table chunks); layer-1 messages are pre-gathered on the host.

One uniform SPMD program runs on all 8 cores: a fixed tile->column-window
schedule makes the instruction stream core-independent; all per-core
variation flows through data (indices, dstcol, pool/norm vectors).
"""
import sys
sys.path.insert(0, '/opt/trn_rl_repo')

import numpy as np
from ml_dtypes import bfloat16 as ml_bf16

import concourse.bass as bass
import concourse.bacc as bacc
import concourse.tile as tile
import concourse.mybir as mybir
from concourse import bass_utils
from concourse.masks import make_identity

F32 = mybir.dt.float32
I16 = mybir.dt.int16
BF16 = mybir.dt.bfloat16

N_CORES = 8
SBATCH = 8  # S tiles built per DVE op
NQ = 4  # SWDGE queues: gather on queue q runs on gpsimd cpu pair (2q, 2q+1)

# -------------------- host-side graph preprocessing --------------------


SEG = 512


def colbase_of(t, B, T, ncols=128):
    """Per-512-segment schedule: T = nseg*T_seg tiles; within a segment the
    window base sweeps [0, SEG-ncols] linearly (no PSUM bank straddles)."""
    ss = min(SEG, B)
    nseg = B // ss
    T_seg = T // nseg
    seg, ts = divmod(t, T_seg)
    return ss * seg + ((ss - ncols) * ts) // max(T_seg - 1, 1)


def _feasible(dst, T, B, ncols=128, cap=128):
    if len(dst) == 0:
        return True
    if len(dst) > T * cap:
        return False
    ptr, n = 0, len(dst)
    for t in range(T):
        base = colbase_of(t, B, T, ncols)
        end = base + ncols
        c = 0
        while ptr < n and c < cap and dst[ptr] < end:
            if dst[ptr] < base:
                return False
            ptr += 1
            c += 1
    return ptr == n


def _feasible_T(runs, B, ncols=128, cap=128):
    """Min per-run T (in nseg steps): runs is a list of dst lists for ONE
    run index across cores; returns smallest T covering all of them."""
    nseg = B // min(SEG, B)
    Ts = 1
    for dst in runs:
        while not _feasible(dst, Ts * nseg, B, ncols, cap):
            Ts += 1
    return Ts * nseg


def _greedy_cbs(dsts, B, ncols=128, cap=128):
    """Shared greedy window schedule for one run: dsts is one sorted dst list
    per core. Each tile's window base is the min pending dst across cores
    (clamped to fit); every core consumes up to cap edges inside the window.
    Returns the list of window bases (one per tile)."""
    ptrs = [0] * len(dsts)
    cbs = []
    while True:
        pend = [d[p] for d, p in zip(dsts, ptrs) if p < len(d)]
        if not pend:
            break
        cb = min(int(min(pend)), B - ncols)
        for i, d in enumerate(dsts):
            p, c = ptrs[i], 0
            while p < len(d) and c < cap and d[p] < cb + ncols:
                p += 1
                c += 1
            ptrs[i] = p
        cbs.append(cb)
    return cbs or [0]


def _assign_tiles_cbs(dst, cbs, cap=128):
    """Walk one core's sorted dst list through the shared cb schedule."""
    out = [[] for _ in cbs]
    ptr, n = 0, len(dst)
    for t, cb in enumerate(cbs):
        while ptr < n and len(out[t]) < cap and dst[ptr] < cb + 128:
            assert dst[ptr] >= cb
            out[t].append(ptr)
            ptr += 1
    assert ptr == n, "greedy schedule infeasible"
    return out


def _assign_tiles(dst, B, T, ncols=128, cap=128):
    out = [[] for _ in range(T)]
    ptr, n = 0, len(dst)
    for t in range(T):
        base = colbase_of(t, B, T, ncols)
        end = base + ncols
        while ptr < n and len(out[t]) < cap and dst[ptr] < end:
            assert dst[ptr] >= base
            out[t].append(ptr)
            ptr += 1
    assert ptr == n, f"schedule infeasible T={T}"
    return out


def prep_host(x, src, dst, graph_ids, n_graphs, B, nblk):
    N = x.shape[0]
    deg_out = np.bincount(src, minlength=N).astype(np.float32)
    deg_in = np.bincount(dst, minlength=N).astype(np.float32)
    norm_out = 1.0 / np.sqrt(np.maximum(deg_out, 1.0))
    norm_in = 1.0 / np.sqrt(np.maximum(deg_in, 1.0))

    gstart = np.searchsorted(graph_ids, np.arange(n_graphs + 1))
    cuts = [0]
    for c in range(1, N_CORES):
        g = int(np.searchsorted(gstart, N * c // N_CORES))
        cuts.append(min(max(g, cuts[-1]), n_graphs))
    cuts.append(n_graphs)
    core_glo, core_ghi = cuts[:-1], cuts[1:]
    core_nlo = [int(gstart[g]) for g in core_glo]
    core_nhi = [int(gstart[g]) for g in core_ghi]

    Npad = B * nblk
    maxn = max(hi - lo for lo, hi in zip(core_nlo, core_nhi))
    assert maxn <= Npad, f"core nodes {maxn} > Npad {Npad}"

    # per-core relabel: deal nodes to blocks round-robin by in-degree rank
    # (balances block degree mass), then place randomly within the block so
    # any 128-position window sees near-uniform edge density
    rng = np.random.default_rng(12345)
    pos_global = np.zeros(N, dtype=np.int64)
    inv_pos = []
    for c in range(N_CORES):
        lo, hi = core_nlo[c], core_nhi[c]
        order = np.argsort(-deg_in[lo:hi], kind='stable')
        inv = np.full(Npad, -1, dtype=np.int64)
        blocks = [[] for _ in range(nblk)]
        for i, nd in enumerate(order):
            blocks[i % nblk].append(nd)
        for b in range(nblk):
            lst = blocks[b]
            perm = rng.permutation(B)[:len(lst)]
            for nd, p0 in zip(lst, perm):
                p = b * B + int(p0)
                pos_global[lo + nd] = p
                inv[p] = nd
        inv_pos.append(inv)

    owner = np.searchsorted(np.array(core_nhi), np.arange(N), side='right')
    dst_owner = owner[dst]

    NTOT = N_CORES * Npad
    CH = 4
    QS = Npad // CH  # quarter rows per core
    assert QS % 128 == 0, f"Npad/CH={QS} must be a multiple of 128"
    chunk_rows = NTOT // CH
    assert chunk_rows <= 32767, f"chunk_rows {chunk_rows} exceeds int16"
    # table rows are quarter-major: chunk q holds [core, pos % QS] for
    # positions in [q*QS, (q+1)*QS); local row within chunk:
    src_pos = pos_global[src]
    src_chunk = src_pos // QS
    src_lrow = owner[src] * QS + (src_pos % QS)

    cores = []
    per_core_runs = []
    for c in range(N_CORES):
        mask = dst_owner == c
        e_lrow = src_lrow[mask]
        e_dstpos = pos_global[dst[mask]]
        e_srcorig = src[mask]
        blk = e_dstpos // B
        chunk = src_chunk[mask]
        order = np.lexsort((e_dstpos, chunk, blk))
        e_lrow, e_dstpos = e_lrow[order], e_dstpos[order]
        e_srcorig = e_srcorig[order]
        blk, chunk = blk[order], chunk[order]
        runs = {}
        for b in range(nblk):
            for ch in range(CH):
                idxs = np.nonzero((blk == b) & (chunk == ch))[0]
                dl = (e_dstpos[idxs] - b * B).astype(np.int64)
                runs[(b, ch)] = (idxs, dl)
        per_core_runs.append(runs)
        cores.append(dict(glo=core_glo[c], ghi=core_ghi[c],
                          nlo=core_nlo[c], nhi=core_nhi[c],
                          e_lrow=e_lrow, e_srcorig=e_srcorig))

    nruns = nblk * CH
    # per-run greedy window schedule shared by all cores
    cb_lists = []
    for b in range(nblk):
        for ch in range(CH):
            cb_lists.append(_greedy_cbs(
                [per_core_runs[c][(b, ch)][1] for c in range(N_CORES)], B))
    T_list = [len(cbs) for cbs in cb_lists]
    off = np.concatenate([[0], np.cumsum(T_list)]).astype(np.int64)
    TT = int(off[-1])

    xs = x * norm_out[:, None]  # y1 = x * norm_out (layer-1 table, host side)
    for c in range(N_CORES):
        runs = per_core_runs[c]
        c0 = cores[c]
        idx16 = np.zeros(TT * 128, np.int16)
        dstcol = np.full(TT * 128, -1.0, np.float32)
        msrc = np.zeros(TT * 128, np.int64)
        mreal = np.zeros(TT * 128, bool)
        for b in range(nblk):
            for ch in range(CH):
                ridx = b * CH + ch
                o = int(off[ridx])
                eidx, dl = runs[(b, ch)]
                cbs = cb_lists[ridx]
                tiles = _assign_tiles_cbs(dl, cbs)
                for t, members in enumerate(tiles):
                    cb = cbs[t]
                    for k, m in enumerate(members):
                        s = (o + t) * 128 + k
                        ge = eidx[m]
                        idx16[s] = c0['e_lrow'][ge]
                        dstcol[s] = dl[m] - cb
                        msrc[s] = c0['e_srcorig'][ge]
                        mreal[s] = True

        idx_dev = np.zeros((128, TT * 8), np.int16)
        for r in range(nruns):
            o, T_r = int(off[r]), T_list[r]
            a = idx16[o * 128:(o + T_r) * 128].reshape(T_r * 8, 16).T
            idx_dev[:, o * 8:(o + T_r) * 8] = np.tile(a, (8, 1))
        c0['idx_dev'] = idx_dev
        c0['dstcol_dev'] = np.ascontiguousarray(
            dstcol.reshape(TT, 128).T).astype(ml_bf16)
        msg = xs[msrc].astype(np.float32)
        msg[~mreal, :] = 0.0
        msg = msg.astype(ml_bf16).reshape(TT, 128, 64).transpose(1, 0, 2)
        c0['msg1_dev'] = np.ascontiguousarray(msg.reshape(128, TT * 64))

        # per-position vectors (position space, pads harmless)
        inv = inv_pos[c]
        lo = c0['nlo']
        ni_pos = np.ones(Npad, np.float32)
        no_pos = np.zeros(Npad, np.float32)
        valid = inv >= 0
        ni_pos[valid] = norm_in[lo + inv[valid]]
        no_pos[valid] = norm_out[lo + inv[valid]]
        c0['invni_dev'] = (1.0 / ni_pos).reshape(1, Npad)
        c0['nio_dev'] = np.ascontiguousarray(
            (ni_pos * no_pos).reshape(Npad // 128, 128).T)  # [128, Npad/128]
        # pooling matrix with norm_in folded: P'[p, g] = norm_in[p]/n_g
        P = np.zeros((Npad, 128), np.float32)
        counts = (gstart[1:] - gstart[:-1]).astype(np.float64)
        gl = c0['glo']
        pidx = np.nonzero(valid)[0]
        gg = graph_ids[lo + inv[pidx]]
        P[pidx, gg - gl] = ni_pos[pidx] / np.maximum(counts[gg], 1.0)
        c0['pool_dev'] = P
    return dict(T_list=T_list, cb_lists=cb_lists, CH=CH,
                chunk_rows=chunk_rows, Npad=Npad, NTOT=NTOT, cores=cores)


# -------------------- device program --------------------


def build_program(T_list, cb_lists, CH, chunk_rows, Npad, NTOT, B, nblk):
    W_IN, W1, W2, W3, W4 = 64, 128, 64, 32, 4
    nruns = nblk * CH
    njt = B // 128  # node-major tiles per block
    off = [0]
    for t in T_list:
        off.append(off[-1] + t)
    TT = off[-1]
    Tmax = max(T_list)
    nc = bacc.Bacc("TRN2", target_bir_lowering=False, debug=False,
                   num_devices=N_CORES, num_swdge_queues=NQ)

    msg1_d = nc.dram_tensor("msg1", [128, TT * 64], BF16, kind="ExternalInput")
    idx_d = nc.dram_tensor("idx", [128, TT * 8], I16, kind="ExternalInput")
    dstcol_d = nc.dram_tensor("dstcol", [128, TT], BF16, kind="ExternalInput")
    w1_d = nc.dram_tensor("W1", [W_IN, W1], BF16, kind="ExternalInput")
    w2_d = nc.dram_tensor("W2", [W1, W2], BF16, kind="ExternalInput")
    w3_d = nc.dram_tensor("W3", [W2, W3], BF16, kind="ExternalInput")
    w4_d = nc.dram_tensor("W4", [W3, W4], BF16, kind="ExternalInput")
    b1_d = nc.dram_tensor("b1", [1, W1], BF16, kind="ExternalInput")
    b2_d = nc.dram_tensor("b2", [1, W2], BF16, kind="ExternalInput")
    b3_d = nc.dram_tensor("b3", [1, W3], BF16, kind="ExternalInput")
    b4_d = nc.dram_tensor("b4", [1, W4], BF16, kind="ExternalInput")
    invni_d = nc.dram_tensor("invni", [1, Npad], BF16, kind="ExternalInput")
    nio_d = nc.dram_tensor("nio", [128, Npad // 128], F32, kind="ExternalInput")
    pool_d = nc.dram_tensor("pool", [Npad, 128], BF16, kind="ExternalInput")
    out_d = nc.dram_tensor("out", [128, W4], F32, kind="ExternalOutput")

    with tile.TileContext(nc) as tc:
        import contextlib
        ctx = contextlib.ExitStack()
        with ctx:
            const = ctx.enter_context(tc.tile_pool(name="const", bufs=1))
            meta = ctx.enter_context(tc.tile_pool(name="meta", bufs=1))
            slabs = ctx.enter_context(tc.tile_pool(name="slabs", bufs=13))
            spool = ctx.enter_context(tc.tile_pool(name="spool", bufs=3))
            hpool = ctx.enter_context(tc.tile_pool(name="hpool", bufs=1))
            nmpool = ctx.enter_context(tc.tile_pool(name="nmpool", bufs=3))
            ppool = ctx.enter_context(tc.tile_pool(name="ppool", bufs=3))
            psum_agg = ctx.enter_context(tc.tile_pool(name="psA", bufs=2, space="PSUM"))
            psum_w = ctx.enter_context(tc.tile_pool(name="psW", bufs=1, space="PSUM"))
            psum_tr = ctx.enter_context(tc.tile_pool(name="psT", bufs=2, space="PSUM"))
            psum_pool = ctx.enter_context(tc.tile_pool(name="psP", bufs=1, space="PSUM"))
            dram = ctx.enter_context(tc.tile_pool(name="dram", bufs=1, space="DRAM"))

            iota_i = const.tile([128, SBATCH * 128], mybir.dt.int32)
            nc.gpsimd.iota(iota_i[:], pattern=[[0, SBATCH], [1, 128]], base=0,
                           channel_multiplier=0)
            iota = const.tile([128, SBATCH * 128], BF16)
            nc.vector.tensor_copy(iota[:], iota_i[:])
            zerosb = const.tile([1, 512], BF16)
            nc.vector.memset(zerosb[:], 0.0)
            ident_f = const.tile([128, 128], F32)
            make_identity(nc, ident_f[:])
            ident = const.tile([128, 128], BF16)
            nc.vector.tensor_copy(ident[:], ident_f[:])

            w1_t = const.tile([W_IN, W1], BF16)
            w2_t = const.tile([W1, W2], BF16)
            w3_t = const.tile([W2, W3], BF16)
            w4_t = const.tile([W3, W4], BF16)
            for wt, wd in ((w1_t, w1_d), (w2_t, w2_d), (w3_t, w3_d), (w4_t, w4_d)):
                nc.sync.dma_start(out=wt[:], in_=wd[:, :])
            b1_t = const.tile([1, W1], BF16)
            b2_t = const.tile([1, W2], BF16)
            b3_t = const.tile([1, W3], BF16)
            b4_t = const.tile([1, W4], BF16)
            for bt, bd in ((b1_t, b1_d), (b2_t, b2_d), (b3_t, b3_d), (b4_t, b4_d)):
                nc.sync.dma_start(out=bt[:], in_=bd[:, :])
            invni_t = const.tile([1, Npad], BF16)
            nc.sync.dma_start(out=invni_t[:], in_=invni_d[:, :])
            nio_t = const.tile([128, Npad // 128], F32)
            nc.sync.dma_start(out=nio_t[:], in_=nio_d[:, :])

            idx_t = meta.tile([128, TT * 8], I16)
            nc.sync.dma_start(out=idx_t[:], in_=idx_d[:, :])
            dstcol_t = meta.tile([128, TT], BF16)
            nc.sync.dma_start(out=dstcol_t[:], in_=dstcol_d[:, :])

            QS = Npad // CH
            TW = (W2, W3, W3)  # true data widths of the three tables
            tables = [[dram.tile([N_CORES * QS, 128], BF16,
                                 tag=f"table{i}_{q}", name=f"table{i}_{q}",
                                 addr_space="Shared")
                       for q in range(CH)] for i in range(3)]
            tloc = [[dram.tile([QS, 128], BF16, tag=f"tloc{i}_{q}",
                               name=f"tloc{i}_{q}")
                     for q in range(CH)] for i in range(3)]

            def emit_ag(i):
                for q in range(CH):
                    nc.gpsimd.collective_compute(
                        "AllGather", mybir.AluOpType.bypass,
                        ins=[tloc[i][q].opt()], outs=[tables[i][q].opt()],
                        replica_groups=[list(range(N_CORES))])

            ACT = mybir.ActivationFunctionType
            AL = mybir.AluOpType

            def agg_layer(get_slab, bias_row, bias_w, evac, lw=128):
                """Aggregate one layer; psum gets raw z (+ bias*invni if
                bias_row); evac(b, ps). lw = slab slot width."""
                for b in range(nblk):
                    slab_tiles = [get_slab(b, chv) for chv in range(CH)]
                    ps = psum_agg.tile([128, B], F32)
                    o = 0
                    while o < B:
                        n = min(512, B - o)
                        nc.tensor.matmul(out=ps[:, o:o + n], lhsT=zerosb[:1, :128],
                                         rhs=zerosb[:1, :n], start=True, stop=False,
                                         skip_group_check=True)
                        o += n
                    for chv in range(CH):
                        slab = slab_tiles[chv]
                        ridx = b * CH + chv
                        T_r = T_list[ridx]
                        for t0 in range(0, T_r, SBATCH):
                            nb = min(SBATCH, T_r - t0)
                            col = off[ridx] + t0
                            s_t = spool.tile([128, SBATCH * 128], BF16, tag="S")
                            dc = dstcol_t[:, col:col + nb]
                            dc = dc.rearrange("p (n o) -> p n o", o=1)
                            nc.vector.tensor_tensor(
                                out=s_t[:, :nb * 128].rearrange(
                                    "p (n c) -> p n c", c=128),
                                in0=iota[:, :nb * 128].rearrange(
                                    "p (n c) -> p n c", c=128),
                                in1=dc.to_broadcast([128, nb, 128]),
                                op=AL.is_equal)
                            for ti in range(nb):
                                t = t0 + ti
                                cb = cb_lists[ridx][t]
                                last = (chv == CH - 1) and (t == T_r - 1) \
                                    and bias_row is None
                                m = min(128, 512 - cb % 512)
                                nc.tensor.matmul(
                                    out=ps[:lw, cb:cb + m],
                                    lhsT=slab[:, t * lw:(t + 1) * lw],
                                    rhs=s_t[:, ti * 128:ti * 128 + m],
                                    start=False, stop=last and m == 128,
                                    skip_group_check=True)
                                if m < 128:
                                    nc.tensor.matmul(
                                        out=ps[:lw, cb + m:cb + 128],
                                        lhsT=slab[:, t * lw:(t + 1) * lw],
                                        rhs=s_t[:, ti * 128 + m:(ti + 1) * 128],
                                        start=False, stop=last,
                                        skip_group_check=True)
                    if bias_row is not None:
                        o = 0
                        while o < B:
                            n = min(512, B - o)
                            lastc = o + n >= B
                            nc.tensor.matmul(out=ps[:bias_w, o:o + n],
                                             lhsT=bias_row,
                                             rhs=invni_t[:, b * B + o:b * B + o + n],
                                             start=False, stop=lastc,
                                             skip_group_check=True)
                            o += n
                    evac(b, ps)

            def transpose_scale_store(b, vT, w, tab_idx):
                """vT [w, B] FM -> node-major [128, 128] tiles scaled by nio
                -> tloc[tab_idx] (cols w: stay stale junk; they only ever land
                in unused psum rows downstream), one DMA per chunk-run."""
                nmw = nmpool.tile([128, njt * 128], BF16, tag="nmw")
                for j in range(njt):
                    pt = psum_tr.tile([128, 64], BF16, tag="pt")
                    nc.tensor.transpose(out=pt[:, :w],
                                        in_=vT[:w, j * 128:(j + 1) * 128],
                                        identity=ident[:w, :w])
                    jg = b * njt + j
                    nc.scalar.activation(nmw[:, j * 128:j * 128 + w], pt[:, :w],
                                         ACT.Copy, scale=nio_t[:, jg:jg + 1])
                j0 = 0
                while j0 < njt:
                    row = b * B + j0 * 128
                    q, qr = divmod(row, QS)
                    jn = min(njt - j0, (QS - qr) // 128)
                    nc.sync.dma_start(
                        out=tloc[tab_idx][q][qr:qr + jn * 128, :].rearrange(
                            "(j p) c -> p j c", p=128),
                        in_=nmw[:, j0 * 128:(j0 + jn) * 128].rearrange(
                            "p (j c) -> p j c", c=128))
                    j0 += jn

            def fm_matmul(rT, w_in, w_out, w_tile, bias_row, act, out_tag, b):
                """out = act(W^T @ rT [+ bias x invni]) -> [w_out, B] sbuf."""
                oT = hpool.tile([w_out, B], BF16, tag=out_tag, name=f"oT_{out_tag}")
                o = 0
                while o < B:
                    n = min(512, B - o)
                    pw = psum_w.tile([w_out, 512], F32, tag="pw")
                    nc.tensor.matmul(out=pw[:, :n], lhsT=w_tile[:],
                                     rhs=rT[:w_in, o:o + n], start=True,
                                     stop=bias_row is None, skip_group_check=True)
                    if bias_row is not None:
                        nc.tensor.matmul(out=pw[:, :n], lhsT=bias_row,
                                         rhs=invni_t[:, b * B + o:b * B + o + n],
                                         start=False, stop=True,
                                         skip_group_check=True)
                    nc.scalar.activation(oT[:, o:o + n], pw[:, :n], act)
                    o += n
                return oT

            # ---------------- Layer 1 ----------------
            def l1_slab(b, chv):
                ridx = b * CH + chv
                T_r = T_list[ridx]
                sl = slabs.tile([128, Tmax * 128], BF16, tag="slab")
                nc.sync.dma_start(
                    out=sl[:, :T_r * 64],
                    in_=msg1_d[:, off[ridx] * 64:(off[ridx] + T_r) * 64])
                return sl

            def l1_evac(b, ps):
                z1T = hpool.tile([64, B], BF16, tag="zT")
                nc.scalar.activation(z1T[:], ps[:64, :], ACT.Copy)
                r1 = fm_matmul(z1T, 64, W1, w1_t, b1_t[:1, :], ACT.Relu, "r", b)
                v2 = fm_matmul(r1, W1, W2, w2_t, None, ACT.Copy, "v", b)
                transpose_scale_store(b, v2, W2, 0)

            agg_layer(l1_slab, None, 0, l1_evac, lw=64)
            emit_ag(0)

            def gather_slab(tab, b, chv):
                ridx = b * CH + chv
                T_r = T_list[ridx]
                sl = slabs.tile([128, Tmax * 128], BF16, tag="slab")
                nc.gpsimd.dma_gather(
                    out_ap=sl[:, :T_r * 128].rearrange("p (s d) -> p s d", d=128),
                    in_ap=tab[chv][:, :],
                    idxs_ap=idx_t[:, off[ridx] * 8:(off[ridx] + T_r) * 8],
                    num_idxs=T_r * 128, num_idxs_reg=T_r * 128,
                    elem_size=128, single_packet=False,
                    queue_num=chv % NQ)
                return sl

            # ---------------- Layer 2 ----------------
            def l2_evac(b, ps):
                r2 = hpool.tile([W2, B], BF16, tag="r", name="r2")
                nc.scalar.activation(r2[:], ps[:W2, :], ACT.Relu)
                v3 = fm_matmul(r2, W2, W3, w3_t, None, ACT.Copy, "v", b)
                transpose_scale_store(b, v3, W3, 1)

            agg_layer(lambda b, chv: gather_slab(tables[0], b, chv),
                      b2_t[:1, :], W2, l2_evac)
            emit_ag(1)

            # ---------------- Layer 3 ----------------
            def l3_evac(b, ps):
                r3 = hpool.tile([W3, B], BF16, tag="r", name="r3")
                nc.scalar.activation(r3[:], ps[:W3, :], ACT.Relu)
                transpose_scale_store(b, r3, W3, 2)

            agg_layer(lambda b, chv: gather_slab(tables[1], b, chv),
                      b3_t[:1, :], W3, l3_evac)
            emit_ag(2)

            # ---------------- Layer 4 + pooling ----------------
            pp = psum_pool.tile([128, W4], F32)

            def l4_evac(b, ps):
                z4T = hpool.tile([W3, B], BF16, tag="zT", name="z4T")
                nc.scalar.activation(z4T[:], ps[:W3, :], ACT.Copy)
                r4 = fm_matmul(z4T, W3, W4, w4_t, b4_t[:1, :], ACT.Copy, "r", b)
                pmw = ppool.tile([128, njt * 128], BF16, tag="poolmat")
                nc.sync.dma_start(
                    out=pmw[:].rearrange("p (j c) -> p j c", c=128),
                    in_=pool_d[b * B:(b + 1) * B, :].rearrange(
                        "(j p) c -> p j c", p=128))
                for j in range(njt):
                    pt = psum_tr.tile([128, 64], BF16, tag="pt")
                    nc.tensor.transpose(out=pt[:, :W4],
                                        in_=r4[:W4, j * 128:(j + 1) * 128],
                                        identity=ident[:W4, :W4])
                    nm = nmpool.tile([128, 64], BF16, tag="nm4")
                    nc.scalar.activation(nm[:, :W4], pt[:, :W4], ACT.Copy)
                    jg = b * njt + j
                    nc.tensor.matmul(out=pp[:], lhsT=pmw[:, j * 128:(j + 1) * 128],
                                     rhs=nm[:, :W4],
                                     start=(jg == 0), stop=(jg == nblk * njt - 1),
                                     skip_group_check=True)

            agg_layer(lambda b, chv: gather_slab(tables[2], b, chv),
                      None, 0, l4_evac)

            outp = ppool.tile([128, W4], F32, tag="outp")
            nc.scalar.activation(outp[:], pp[:], ACT.Copy)
            nc.sync.dma_start(out=out_d[:, :], in_=outp[:])

    nc.compile()
    return nc


# -------------------- top-level kernel --------------------


def _run(x, W1, b1, W2, b2, W3, b3, W4, b4, src, dst, graph_ids,
         n_graphs, B, nblk):
    x = np.asarray(x, np.float32)
    src = np.asarray(src, np.int64)
    dst = np.asarray(dst, np.int64)
    graph_ids = np.asarray(graph_ids, np.int64)
    H = prep_host(x, src, dst, graph_ids, n_graphs, B, nblk)
    import os as _os
    if _os.environ.get('GCN_DEBUG'):
        TT = sum(H['T_list'])
        print(f"[kernel] TT={TT} slots={TT * 128} Tmax={max(H['T_list'])} "
              f"Npad={H['Npad']} NTOT={H['NTOT']}")
    nc = build_program(H['T_list'], H['cb_lists'], H['CH'], H['chunk_rows'],
                       H['Npad'], H['NTOT'], B, nblk)
    in_maps = []
    for c in range(N_CORES):
        c0 = H['cores'][c]
        in_maps.append({
            "msg1": c0['msg1_dev'],
            "idx": c0['idx_dev'],
            "dstcol": c0['dstcol_dev'],
            "W1": np.asarray(W1, np.float32).astype(ml_bf16),
            "W2": np.asarray(W2, np.float32).astype(ml_bf16),
            "W3": np.asarray(W3, np.float32).astype(ml_bf16),
            "W4": np.asarray(W4, np.float32).astype(ml_bf16),
            "b1": np.asarray(b1, np.float32).reshape(1, -1).astype(ml_bf16),
            "b2": np.asarray(b2, np.float32).reshape(1, -1).astype(ml_bf16),
            "b3": np.asarray(b3, np.float32).reshape(1, -1).astype(ml_bf16),
            "b4": np.asarray(b4, np.float32).reshape(1, -1).astype(ml_bf16),
            "invni": c0['invni_dev'].astype(ml_bf16),
            "nio": c0['nio_dev'],
            "pool": c0['pool_dev'].astype(ml_bf16),
        })
    res = bass_utils.run_bass_kernel_spmd(
        nc, in_maps, core_ids=list(range(N_CORES)),
        trace=bool(int(__import__('os').environ.get('GCN_TRACE', '0'))))
    out = np.zeros((n_graphs, 4), np.float32)
    for c in range(N_CORES):
        c0 = H['cores'][c]
        g0, g1 = c0['glo'], c0['ghi']
        out[g0:g1] = res.results[c]["out"][:g1 - g0, :]
    _run.last_exec_ns = res.exec_time_ns
    _run.last_res = res
    return out


def kernel(x, W1, b1, W2, b2, W3, b3, W4, b4, src, dst, graph_ids):
    return _run(x, W1, b1, W2, b2, W3, b3, W4, b4, src, dst, graph_ids,
                n_graphs=500, B=1024, nblk=14)

